# revision 28
# baseline (speedup 1.0000x reference)
"""AttentionBlock (GroupNorm -> qkv conv1x1 -> 4-head attention -> proj -> residual)
as a distributed Bass/Tile kernel on 8 TRN2 NeuronCores.

Sharding: core j handles batch b = j//2 and query-half h = j%2. The host
permutes x's spatial columns per core so queries are always cols 0:2048
(attention is permutation-invariant over keys). K/V are computed full-length
per core, so output slices are disjoint and no collectives are needed.

Engine balance: softmax exp is split between ScalarE (real Exp on even key
chunks) and VectorE (Schraudolph bit-trick exp on odd chunks: bf16 bit
pattern of round(a*s+b) int16 ~= exp(s*scale), ~3% pointwise, ~4e-4 end to
end because the residual dominates). 1/Z via scalar Ln->Exp(-x). GroupNorm
rstd via exp(-0.5*ln(var+eps)). Residual adds and the bf16 cast of x run on
GpSimd. Conv weights are rstd-folded on device so PE conv work starts
without waiting for GN statistics to be applied to x.
"""
import numpy as np
import ml_dtypes

import concourse.bass as bass
import concourse.bacc as bacc
import concourse.tile as tile
from concourse import mybir
from concourse import bass_utils
from concourse.bass_interp import get_hw_module

F32 = mybir.dt.float32
BF16 = mybir.dt.bfloat16
I16 = mybir.dt.int16
BF = ml_dtypes.bfloat16

B, C, Himg, Wimg = 4, 256, 64, 64
T = Himg * Wimg            # 4096 tokens
HEADS, D = 4, 64           # 4 heads x 64 dims
GROUPS, GS = 32, 8         # groupnorm: 32 groups of 8 channels
EPS = 1e-5
TQ = T // 2                # queries per core (2048)
NTT = TQ // 512            # query tiles of 512
NSC = T // 128             # key chunks of 128
SCALE = 1.0 / np.sqrt(D)
A_SCHR = float(128.0 * np.log2(np.e) * SCALE)   # schraudolph mult
B_SCHR = float(127.0 * 128.0 - 5.08)            # schraudolph bias (mean-centering)
Exp = mybir.ActivationFunctionType.Exp
Ln = mybir.ActivationFunctionType.Ln
Lrelu = mybir.ActivationFunctionType.Lrelu

_CACHED = {}


def _patch_act_tables():
    """Restrict the act-table chooser to natural_log_exp_and_others so the
    scalar engine never reloads tables (exp+ln live in one set; identity
    copies are expressed as Lrelu(alpha=1), also in that set). Set order is
    preserved so act_func_set_id stays aligned with act_info.json."""
    if getattr(bacc, "_act_tables_patched", False):
        return
    orig = bacc.get_activation_tables

    def patched(arch):
        t = orig(arch)
        return {name: (fns if name == "natural_log_exp_and_others" else set())
                for name, fns in t.items()}

    bacc.get_activation_tables = patched
    bacc._act_tables_patched = True


def _build():
    _patch_act_tables()
    nc = bacc.Bacc("TRN2", target_bir_lowering=False, debug=False,
                   enable_asserts=False, num_devices=8)

    xb_d = nc.dram_tensor("xb", [C, T], BF16, kind="ExternalInput")
    qkvT_d = nc.dram_tensor("qkvT", [C, 3 * C], BF16, kind="ExternalInput")
    qkvb_d = nc.dram_tensor("qkvb", [3 * C, 1], F32, kind="ExternalInput")
    projT_d = nc.dram_tensor("projT", [HEADS, D, C], BF16, kind="ExternalInput")
    projb_d = nc.dram_tensor("projb", [C, 1], F32, kind="ExternalInput")
    gmat_d = nc.dram_tensor("gmat", [128, 16], F32, kind="ExternalInput")
    gmatT_d = nc.dram_tensor("gmatT", [16, 128], F32, kind="ExternalInput")
    out_d = nc.dram_tensor("out", [C, TQ], F32, kind="ExternalOutput")

    with tile.TileContext(nc) as tc:
        with (
            tc.tile_pool(name="consts", bufs=1) as consts,
            tc.tile_pool(name="data", bufs=1) as data,
            tc.tile_pool(name="gn", bufs=1) as gn,
            tc.tile_pool(name="pt", bufs=13) as ppool,
            tc.tile_pool(name="dn", bufs=2) as dn,
            tc.tile_pool(name="ao", bufs=4) as ao,
            tc.tile_pool(name="ah", bufs=1) as ahpool,
            tc.tile_pool(name="ps", bufs=3, space="PSUM") as psum_s,
            tc.tile_pool(name="pa", bufs=1, space="PSUM") as psum_a,
        ):
            # ---------------- tiles ----------------
            # packed weight tiles: one DMA each (issue time on the queue
            # engine is ~0.6us per descriptor, so fewer is better)
            qkvT2 = consts.tile([128, 2, 3 * C], BF16, tag="qkvT2", name="qkvT2")
            qkvT_sb = [qkvT2[:, ct, :] for ct in range(2)]
            projT4 = consts.tile([D, HEADS, C], BF16, tag="projT4", name="projT4")
            projT_sb = [projT4[:, h, :] for h in range(HEADS)]
            b8 = consts.tile([128, 8], F32, tag="b8", name="b8")
            b_in = [b8[:, m:m + 1] for m in range(6)]
            pb_sb = [b8[:, 6 + oc:7 + oc] for oc in range(2)]
            gmat_sb = consts.tile([128, 16], F32, tag="gmat", name="gmat")
            gmatT_sb = consts.tile([16, 128], F32, tag="gmatT", name="gmatT")
            eps_t = gn.tile([16, 1], F32, tag="eps", name="eps")
            nc.vector.memset(eps_t[:], EPS)

            xb_sb, st_sb = [], []
            for ct in range(2):
                xt = data.tile([128, T], BF16, tag=f"xb{ct}", name=f"xb{ct}")
                xb_sb.append(xt)
                st = gn.tile([128, 8, 6], F32, tag=f"st{ct}", name=f"st{ct}")
                st_sb.append(st)

            # ---------------- loads: x chunks first, weights woven -------
            # x is bf16 (host-converted): halves the DMA and feeds convs
            # directly. Chunks round-robin over the three DGE queues so
            # transfers overlap; weights behind them.
            qdma = [nc.sync, nc.scalar, nc.gpsimd]
            for c4 in range(4):
                for ct in range(2):
                    sl = slice(c4 * 1024, (c4 + 1) * 1024)
                    qdma[(2 * c4 + ct) % 3].dma_start(
                        xb_sb[ct][:, sl], xb_d.ap()[ct * 128:(ct + 1) * 128, sl])
            nc.scalar.dma_start(qkvT2[:], qkvT_d.ap()[:].rearrange(
                "(ct p) o -> p ct o", ct=2))
            nc.sync.dma_start(b8[:, 0:6], qkvb_d.ap()[:].rearrange(
                "(m p) one -> p (m one)", m=6))
            nc.sync.dma_start(b8[:, 6:8], projb_d.ap()[:].rearrange(
                "(oc p) one -> p (oc one)", oc=2))
            nc.sync.dma_start(projT4[:], projT_d.ap()[:].rearrange(
                "h d o -> d h o"))
            nc.gpsimd.dma_start(gmat_sb[:], gmat_d.ap()[:])
            nc.gpsimd.dma_start(gmatT_sb[:], gmatT_d.ap()[:])
            # PE warm-up: junk matmuls on the first landed chunk keep the
            # HAM activity monitor busy so the real pipeline starts at
            # 2.4GHz instead of cold 1.2GHz.
            junk_ps = psum_s.tile([128, 512], F32, tag="ps", name="ps")
            for _ in range(96):
                nc.tensor.matmul(junk_ps[:], xb_sb[0][0:128, 0:128],
                                 xb_sb[0][:, 0:512], start=True, stop=True)
            for c4 in range(4):
                for ct in range(2):
                    for half in range(2):
                        sh = slice(c4 * 1024 + half * 512,
                                   c4 * 1024 + (half + 1) * 512)
                        nc.vector.bn_stats(st_sb[ct][:, 2 * c4 + half, :],
                                           xb_sb[ct][:, sh])

            # ---------------- GroupNorm statistics ----------------
            stats2 = []
            for ct in range(2):
                mv = gn.tile([128, 2], F32, tag=f"mv{ct}", name=f"mv{ct}")
                nc.vector.bn_aggr(mv[:], st_sb[ct][:])
                s2 = gn.tile([128, 2], F32, tag=f"s2{ct}", name=f"s2{ct}")
                nc.vector.tensor_copy(s2[:, 0:1], mv[:, 0:1])
                m2 = gn.tile([128, 1], F32, tag=f"m2{ct}", name=f"m2{ct}")
                nc.vector.tensor_mul(m2[:], mv[:, 0:1], mv[:, 0:1])
                nc.vector.tensor_add(s2[:, 1:2], m2[:], mv[:, 1:2])
                stats2.append(s2)

            # group (mean, E[x^2]) -> per-group rstd via exp(-0.5*ln(var+eps))
            gs_ps, bc_sb = [], []
            vg = gn.tile([16, 2], F32, tag="vg", name="vg")
            for ct in range(2):
                g1 = psum_s.tile([16, 2], F32, tag="ps", name="ps")
                nc.tensor.matmul(g1[:], gmat_sb[:], stats2[ct][:],
                                 start=True, stop=True)
                gsb = gn.tile([16, 2], F32, tag=f"gsb{ct}", name=f"gsb{ct}")
                nc.vector.tensor_copy(gsb[:], g1[:])
                gs_ps.append(gsb)
                m2g = gn.tile([16, 1], F32, tag=f"m2g{ct}", name=f"m2g{ct}")
                nc.vector.tensor_mul(m2g[:], gsb[:, 0:1], gsb[:, 0:1])
                nc.vector.tensor_sub(vg[:, ct:ct + 1], gsb[:, 1:2], m2g[:])
            lgv = gn.tile([16, 2], F32, tag="lgv", name="lgv")
            nc.scalar.activation(lgv[:], vg[:], Ln, bias=eps_t[:])
            rg = gn.tile([16, 2], F32, tag="rg", name="rg")
            nc.scalar.activation(rg[:], lgv[:], Exp, scale=-0.5)
            for ct in range(2):
                bcv = gn.tile([16, 2], F32, tag=f"bcv{ct}", name=f"bcv{ct}")
                nc.vector.tensor_copy(bcv[:, 0:1], gs_ps[ct][:, 0:1])
                nc.vector.tensor_copy(bcv[:, 1:2], rg[:, ct:ct + 1])
                b1 = psum_s.tile([128, 2], F32, tag="ps", name="ps")
                nc.tensor.matmul(b1[:], gmatT_sb[:], bcv[:],
                                 start=True, stop=True)
                bsb = gn.tile([128, 2], F32, tag=f"bc{ct}", name=f"bc{ct}")
                nc.vector.tensor_copy(bsb[:], b1[:])
                bc_sb.append(bsb)

            # ---------------- fold rstd into weights; bias b2 = b - W'mu ---
            qkvS_sb, mu_bf = [], []
            for ct in range(2):
                ws = consts.tile([128, 3 * C], BF16, tag=f"qkvS{ct}", name=f"qkvS{ct}")
                nc.vector.tensor_scalar_mul(out=ws[:], in0=qkvT_sb[ct][:],
                                            scalar1=bc_sb[ct][:, 1:2])
                qkvS_sb.append(ws)
                mb = gn.tile([128, 1], BF16, tag=f"mub{ct}", name=f"mub{ct}")
                nc.vector.tensor_copy(mb[:], bc_sb[ct][:, 0:1])
                mu_bf.append(mb)
            b2_sb = []
            for m in range(6):
                wm = psum_s.tile([128, 1], F32, tag="ps", name="ps")
                for ct in range(2):
                    nc.tensor.matmul(wm[:], qkvS_sb[ct][:, m * 128:(m + 1) * 128],
                                     mu_bf[ct][:], start=(ct == 0), stop=(ct == 1))
                b2 = gn.tile([128, 1], F32, tag=f"b2_{m}", name=f"b2_{m}")
                nc.vector.tensor_sub(b2[:], b_in[m][:], wm[:])
                b2_sb.append(b2)
            # bf16 copies of the v-slice biases for the proj-bias fold;
            # odd heads live at partitions 64-127 -> DMA down to base 0.
            bv_h = [None] * HEADS
            for m in (4, 5):
                bb = gn.tile([128, 1], BF16, tag=f"bv{m}", name=f"bv{m}")
                nc.vector.tensor_copy(bb[:], b2_sb[m][:])
                bv_h[2 * (m - 4)] = bb[0:64, :]
                lo = gn.tile([64, 1], BF16, tag=f"bvl{m}", name=f"bvl{m}")
                nc.sync.dma_start(lo[:], bb[64:128, :])
                bv_h[2 * (m - 4) + 1] = lo[:]

            # ---------------- SBUF destination tiles ----------------
            k_sb = [data.tile([128, T], BF16, tag=f"k{p}", name=f"k{p}")
                    for p in range(2)]
            q_sb = [data.tile([128, TQ], BF16, tag=f"q{p}", name=f"q{p}")
                    for p in range(2)]
            vt_sb = data.tile([128, NSC * 260], BF16, tag="vt", name="vt")
            ones_cols = vt_sb[:].rearrange("p (s h c) -> p s h c", s=NSC, c=65)
            nc.vector.memset(ones_cols[:, :, :, 64:65], 1.0)

            # pb2[oc] = projb[oc] + sum_h projT_h[:,oc]^T @ bv_h
            pb2_sb = []

            def emit_pb2():
                for oc in range(2):
                    pv = psum_s.tile([128, 1], F32, tag="ps", name="ps")
                    for h in range(HEADS):
                        nc.tensor.matmul(pv[:],
                                         projT_sb[h][:, oc * 128:(oc + 1) * 128],
                                         bv_h[h], start=(h == 0), stop=(h == HEADS - 1))
                    pb2 = gn.tile([128, 1], F32, tag=f"pb2_{oc}", name=f"pb2_{oc}")
                    nc.vector.tensor_add(pb2[:], pb_sb[oc][:], pv[:])
                    pb2_sb.append(pb2)

            # ---------------- conv units (woven into attention) ----------
            def emit_k(p, t8):
                kv = psum_s.tile([128, 512], F32, tag="ps", name="ps")
                for ct in range(2):
                    nc.tensor.matmul(
                        kv[:], qkvS_sb[ct][:, C + p * 128:C + (p + 1) * 128],
                        xb_sb[ct][:, t8 * 512:(t8 + 1) * 512],
                        start=(ct == 0), stop=(ct == 1))
                nc.vector.tensor_scalar_add(
                    out=k_sb[p][:, t8 * 512:(t8 + 1) * 512],
                    in0=kv[:], scalar1=b2_sb[2 + p][:])

            def emit_q(p, t4):
                qp = psum_s.tile([128, 512], F32, tag="ps", name="ps")
                for ct in range(2):
                    nc.tensor.matmul(
                        qp[:], qkvS_sb[ct][:, p * 128:(p + 1) * 128],
                        xb_sb[ct][:, t4 * 512:(t4 + 1) * 512],
                        start=(ct == 0), stop=(ct == 1))
                nc.vector.tensor_scalar_add(
                    out=q_sb[p][:, t4 * 512:(t4 + 1) * 512],
                    in0=qp[:], scalar1=b2_sb[p][:])

            def emit_vt(i, eng):
                vp = psum_s.tile([128, C], F32, tag="ps", name="ps")
                for ct in range(2):
                    nc.tensor.matmul(
                        vp[:], xb_sb[ct][:, i * 128:(i + 1) * 128],
                        qkvS_sb[ct][:, 2 * C:3 * C],
                        start=(ct == 0), stop=(ct == 1))
                dst = vt_sb[:, i * 260:(i + 1) * 260].rearrange(
                    "p (h c) -> p h c", c=65)[:, :, 0:64]
                if eng == "s":
                    nc.scalar.copy(dst, vp[:].rearrange("p (h c) -> p h c", c=64))
                else:
                    nc.vector.tensor_copy(dst, vp[:].rearrange("p (h c) -> p h c", c=64))

            # per-(block, chunk) pre-emit schedule of conv units
            sched = {}

            def add_sched(b, i, fn):
                sched.setdefault((b, i), []).append(fn)

            # K(0,0) and Q(0,0) run before the first S matmul (not scheduled).
            # vt: front-loaded in block0 (PV(i) needs vt[i]); alternate copy engine.
            add_sched(0, 0, lambda: emit_vt(0, "s"))
            add_sched(0, 0, lambda: emit_vt(1, "v"))
            add_sched(0, 0, lambda: emit_vt(2, "s"))
            for j in range(3, 32):
                b0c = (j - 3) // 2 + 1          # chunks 1..15, two vts per chunk
                add_sched(0, b0c, (lambda jj: lambda: emit_vt(
                    jj, "s" if jj % 2 else "v"))(j))
            # K(0, t8) for t8>=1 needed by chunk 4*t8 of block0: emit at t8.
            for t8 in range(1, 8):
                add_sched(0, t8, (lambda t: lambda: emit_k(0, t))(t8))
            # Q(0, t4) for t4 1..3 needed at block t4: emit in prior blocks.
            for t4 in range(1, 4):
                add_sched(t4 - 1, 20, (lambda t: lambda: emit_q(0, t))(t4))
            # K(1, *) + Q(1, *) needed from block 4 on: spread over blocks 1-3.
            for t8 in range(8):
                add_sched(1 + t8 // 3, 8 + 3 * (t8 % 3), (lambda t: lambda: emit_k(1, t))(t8))
            for t4 in range(4):
                add_sched(2, 17 + 3 * t4, (lambda t: lambda: emit_q(1, t))(t4))
            add_sched(0, 16, emit_pb2)

            # ---------------- attention ----------------
            # One global 256-chunk pipeline (8 blocks x 32 key chunks). At
            # global step g: emit S(g) + exp(g) + PV(g - PV_LAG). PV of
            # block b thus finishes ~10 chunks into block b+1 with no flush
            # burst. Right after PV(b,31), a single DVE copy moves the raw
            # accumulator (incl. the Z row) to SBUF, freeing the a_ps bank
            # pair within ~2 chunks (pa bufs=1 suffices); the 1/Z chain
            # (Ln -> Exp -> gpsimd broadcast -> divide) then runs off-PSUM,
            # staggered over the following chunks.
            PV_LAG = 10
            ah_sb = {}
            post = {}          # g -> list of closures, run before chunk g

            def at(g, fn):
                post.setdefault(g, []).append(fn)

            def emit_pv(blk, a_ps, i, p_t):
                p = blk // NTT
                for u in range(2):
                    h = 2 * p + u
                    nc.tensor.matmul(
                        a_ps[:, u * 512:(u + 1) * 512],
                        vt_sb[:, i * 260 + h * 65:i * 260 + h * 65 + 65],
                        p_t[:, u * 512:(u + 1) * 512],
                        start=(i == 0), stop=(i == NSC - 1))

            def emit_drain(blk, a_ps, g):
                p, tt = blk // NTT, blk % NTT
                araw = dn.tile([65, 1024], BF16, tag="araw", name="araw")
                nc.vector.tensor_copy(araw[:], a_ps[:])

                def ln_exp():
                    zl = dn.tile([1, 1024], F32, tag="zl", name="zl")
                    nc.scalar.activation(zl[:], araw[64:65, :], Ln)
                    zi = dn.tile([1, 1024], F32, tag="zi", name="zi")
                    nc.scalar.activation(zi[:], zl[:], Exp, scale=-1.0)
                    d_bc = dn.tile([64, 1024], F32, tag="dbc", name="dbc")
                    nc.gpsimd.partition_broadcast(d_bc[:], zi[:])

                    def ah_fin():
                        for u in range(2):
                            h = 2 * p + u
                            ah = ahpool.tile([D, 512], BF16, tag=f"ah{h}_{tt}",
                                             name=f"ah{h}_{tt}")
                            nc.gpsimd.tensor_mul(
                                ah[:], araw[0:64, u * 512:(u + 1) * 512],
                                d_bc[:, u * 512:(u + 1) * 512])
                            ah_sb[(h, tt)] = ah
                    at(g + 12, ah_fin)
                at(g + 2, ln_exp)
                if p == 1:
                    at(g + 16, make_proj(tt))

            def make_proj(tt):
                def proj():
                    for oc in range(2):
                        pr = psum_s.tile([128, 512], F32, tag="ps", name="ps")
                        for h in range(HEADS):
                            nc.tensor.matmul(
                                pr[:], projT_sb[h][:, oc * 128:(oc + 1) * 128],
                                ah_sb[(h, tt)][:],
                                start=(h == 0), stop=(h == HEADS - 1))
                        o1 = ao.tile([128, 512], F32, tag="o1", name="o1")
                        nc.vector.tensor_scalar_add(out=o1[:], in0=pr[:],
                                                    scalar1=pb2_sb[oc][:])
                        o2 = ao.tile([128, 512], F32, tag="o2", name="o2")
                        nc.gpsimd.tensor_add(o2[:], o1[:],
                                             xb_sb[oc][:, tt * 512:(tt + 1) * 512])
                        nc.sync.dma_start(
                            out_d.ap()[oc * 128:(oc + 1) * 128,
                                       tt * 512:(tt + 1) * 512], o2[:])
                return proj

            emit_k(0, 0)
            emit_q(0, 0)

            NG = 2 * NTT * NSC
            pts = {}
            a_cur = {}
            for g in range(NG + PV_LAG):
                for fn in post.pop(g, ()):
                    fn()
                if g < NG:
                    blk, i = g // NSC, g % NSC
                    p, tt = blk // NTT, blk % NTT
                    for fn in sched.pop((blk, i), ()):
                        fn()
                    s_ps = psum_s.tile([128, 1024], F32, tag="ps", name="ps")
                    for u in range(2):
                        nc.tensor.matmul(
                            s_ps[:, u * 512:(u + 1) * 512],
                            k_sb[p][u * 64:(u + 1) * 64, i * 128:(i + 1) * 128],
                            q_sb[p][u * 64:(u + 1) * 64, tt * 512:(tt + 1) * 512],
                            start=True, stop=True,
                            tile_position=(u * 64, 0))
                    p_t = ppool.tile([128, 1024], BF16, tag="pt", name="pt")
                    pts[g] = p_t
                    if i % 2 == 0 or i == 31:
                        nc.scalar.activation(p_t[:], s_ps[:], Exp,
                                             scale=float(SCALE))
                    else:
                        nc.vector.tensor_scalar(
                            out=p_t[:].bitcast(I16), in0=s_ps[:],
                            scalar1=A_SCHR, scalar2=B_SCHR,
                            op0=mybir.AluOpType.mult,
                            op1=mybir.AluOpType.add)
                gp = g - PV_LAG
                if gp >= 0:
                    pblk, pi = gp // NSC, gp % NSC
                    if pi == 0:
                        a_cur[pblk] = psum_a.tile([65, 1024], F32,
                                                  tag="pa", name="pa")
                    emit_pv(pblk, a_cur[pblk], pi, pts.pop(gp))
                    if pi == NSC - 1:
                        emit_drain(pblk, a_cur.pop(pblk), g)
            # tail: keep the PE warm (HAM) while the last drain chain runs,
            # so the final proj matmuls execute at full clock.
            tail_ps = psum_s.tile([128, 512], F32, tag="ps", name="ps")
            for _ in range(20):
                nc.tensor.matmul(tail_ps[:], xb_sb[0][0:128, 0:128],
                                 xb_sb[0][:, 0:512], start=True, stop=True)
            for g in range(NG + PV_LAG, NG + PV_LAG + 16):
                for fn in post.pop(g, ()):
                    fn()
            assert not post and not sched

    nc.compile()
    nc.m = get_hw_module(nc.m)
    return nc


def _host_prep(inputs):
    x = np.asarray(inputs["x"], np.float32)
    gn_w = np.asarray(inputs["gn_weight"], np.float32)
    gn_b = np.asarray(inputs["gn_bias"], np.float32)
    qkv_w = np.asarray(inputs["qkv_w"], np.float32)
    qkv_b = np.asarray(inputs["qkv_b"], np.float32)
    proj_w = np.asarray(inputs["proj_w"], np.float32)
    proj_b = np.asarray(inputs["proj_b"], np.float32)

    W_ = qkv_w * gn_w[None, :]
    b_ = qkv_w @ gn_b + qkv_b
    qkvT = np.ascontiguousarray(W_.T).astype(BF)
    projT = np.ascontiguousarray(proj_w.T.reshape(HEADS, D, C)).astype(BF)

    gmat = np.zeros((128, 16), np.float32)
    gmatT = np.zeros((16, 128), np.float32)
    for ch in range(128):
        gmat[ch, ch // GS] = 1.0 / GS
        gmatT[ch // GS, ch] = 1.0
    shared = {
        "qkvT": qkvT,
        "qkvb": b_.reshape(3 * C, 1).astype(np.float32),
        "projT": projT,
        "projb": proj_b.reshape(C, 1).astype(np.float32),
        "gmat": gmat,
        "gmatT": gmatT,
    }
    x3 = x.reshape(B, C, T).astype(BF)
    in_maps = []
    for j in range(8):
        b, hf = j // 2, j % 2
        m = dict(shared)
        if hf == 0:
            m["xb"] = np.ascontiguousarray(x3[b])
        else:
            m["xb"] = np.ascontiguousarray(
                np.concatenate([x3[b][:, TQ:], x3[b][:, :TQ]], axis=1))
        in_maps.append(m)
    return x, in_maps


def kernel(**inputs) -> np.ndarray:
    if "nc" not in _CACHED:
        _CACHED["nc"] = _build()
    nc = _CACHED["nc"]
    x, in_maps = _host_prep(inputs)
    res = bass_utils.run_bass_kernel_spmd(nc, in_maps, core_ids=list(range(8)))
    out = np.zeros((B, C, T), np.float32)
    for j in range(8):
        b, hf = j // 2, j % 2
        out[b][:, hf * TQ:(hf + 1) * TQ] = np.asarray(
            res.results[j]["out"], np.float32)
    return out.reshape(B, C, Himg, Wimg)
